# revision 1
# baseline (speedup 1.0000x reference)
"""Expert-parallel MoE block (dense path) on 8 Trainium2 NeuronCores.

Reference computation (E=8, C=1024, D_IN=4096, D_OUT=1024, N_TOK=8192):
    expert_out = einsum('eci,eio->eco', expert_input, weight) + bias   # [E,C,D_OUT]
    output     = combine_weights @ expert_out.reshape(E*C, D_OUT)      # [N_TOK,D_OUT]

Sharding (expert-parallel):
  Core e holds expert e: computes X_e = expert_input[e] @ weight[e] + bias[e]
  ([C, D_OUT]); on-device AllGathers assemble X = concat_e X_e ([E*C, D_OUT]);
  core e then computes its token slice of the combine,
      out_e = combine_weights[e*C:(e+1)*C, :] @ X   ([C, D_OUT]),
  and the host concatenates the 8 row blocks.

Performance structure (~440us measured; PE issue-rate floor for the 1536
N=512 matmuls is ~404us because a cc-enabled NEFF runs the PE at ~1.95GHz):
  - Matmul operands are fp16 (PSUM accumulates fp32; measured output
    L2 relative error 2.1e-4). fp16 runs the 128x128 PE at 1 row/cycle --
    same rate as bf16/float32r and 4x faster than exact fp32 -- while
    halving HBM and AllGather bytes.
  - The AllGather is split into 4 token-blocks of decreasing size [384, 256,
    256, 128], each triggered as soon as its block of the expert GEMM
    finishes, so the collectives overlap the remaining expert GEMM.
  - The combine iterates K-column-major (one 128-row k-tile column across
    all 8 experts at a time, in AllGather-block order) with SBUF-resident
    partial-output accumulation, so combine matmuls start as soon as the
    first AllGather lands and never wait for the later ones.
  - Input DMAs are emitted in consumption order (first a/W chunks first) so
    the first matmul issues ~15us into the kernel instead of after the whole
    weight load; expert weights stay SBUF-resident in k-chunks.
  - Host pre-transposes the stationary operands (expert_input and the
    combine-weight row block) so every SBUF operand has the contraction dim
    on partitions.
"""

import numpy as np

E = 8
C = 1024
D_IN = 4096
D_OUT = 1024
N_TOK = E * C
P = 128

KT1 = D_IN // P  # 32 k-tiles in the expert GEMM
BLOCKS = [3, 2, 2, 1]  # expert-GEMM c-blocks in 128-token units; one AG each

_cached = None


def _build():
    import concourse.bass as bass  # noqa: F401
    import concourse.mybir as mybir
    import concourse.tile as tile
    from concourse import bacc

    DT = mybir.dt.float32r
    F16 = mybir.dt.float16
    F32 = mybir.dt.float32

    nc = bacc.Bacc("TRN2", target_bir_lowering=False, debug=False, num_devices=E)

    at = nc.dram_tensor("at", [D_IN, C], F16, kind="ExternalInput").ap()
    w = nc.dram_tensor("w", [D_IN, D_OUT], F16, kind="ExternalInput").ap()
    bias = nc.dram_tensor("bias", [1, D_OUT], DT, kind="ExternalInput").ap()
    cwt = nc.dram_tensor("cwt", [N_TOK, C], F16, kind="ExternalInput").ap()
    out = nc.dram_tensor("out", [C, D_OUT], DT, kind="ExternalOutput").ap()

    NB = len(BLOCKS)
    assert sum(BLOCKS) * P == C
    # Internal DRAM: per-block AllGather bounce buffers (fp16).
    xh = [nc.dram_tensor(f"xh{b}", [BLOCKS[b] * P, D_OUT], F16) for b in range(NB)]
    xg = [
        nc.dram_tensor(f"xg{b}", [E * BLOCKS[b] * P, D_OUT], F16, addr_space="Shared")
        for b in range(NB)
    ]

    at3 = at.rearrange("(ko p) c -> p ko c", p=P)  # [128, 32, 1024]
    w3 = w.rearrange("(ko p) d -> p ko d", p=P)  # [128, 32, 1024]
    cwt3 = cwt.rearrange("(ko p) t -> p ko t", p=P)  # [128, 64, 1024]
    xh3 = [x.rearrange("(ci p) d -> p ci d", p=P) for x in xh]  # [128, S, 1024]
    xg3 = [x.rearrange("(ko p) d -> p ko d", p=P) for x in xg]  # [128, E*S, 1024]
    out4 = out.rearrange("(tb ti p) d -> p tb ti d", p=P, ti=2)  # [128, 4, 2, 1024]

    rg = [list(range(E))]

    with tile.TileContext(nc) as tc:
        # ---------------- phase 1: expert GEMM ----------------
        with (
            tc.tile_pool(name="wpool", bufs=1) as wpool,
            tc.tile_pool(name="apool", bufs=3) as apool,
            tc.tile_pool(name="xepool", bufs=2) as xepool,
            tc.tile_pool(name="biaspool", bufs=1) as biaspool,
            tc.tile_pool(name="ps1", bufs=4, space="PSUM") as ps1,
        ):
            # a-tiles: one per (block, k-quarter); W resident in 4-ktile
            # chunks. Emitted interleaved in rough consumption order so the
            # first matmuls are fed ~15us in.
            SMAX = max(BLOCKS)
            w_cs = [
                wpool.tile([P, 4, D_OUT], F16, tag=f"w{kc}", name=f"w{kc}")
                for kc in range(8)
            ]
            a_ts = {}

            def load_a(b, kq):
                S = BLOCKS[b]
                c0 = sum(BLOCKS[:b]) * P
                t = apool.tile([P, 8, SMAX * P], F16, tag="a", name=f"a_{b}_{kq}")
                nc.sync.dma_start(
                    t[:, :, : S * P],
                    at3[:, kq * 8 : (kq + 1) * 8, c0 : c0 + S * P],
                )
                a_ts[(b, kq)] = t

            load_a(0, 0)
            for kc in range(8):
                nc.sync.dma_start(w_cs[kc][:], w3[:, kc * 4 : (kc + 1) * 4, :])
                if kc % 2 == 1 and kc // 2 + 1 < 4:
                    load_a(0, kc // 2 + 1)
            bias_sb = biaspool.tile([P, D_OUT], DT)
            nc.sync.dma_start(bias_sb[:], bias.to_broadcast((P, D_OUT)))

            for b in range(NB):
                S = BLOCKS[b]
                for kq in range(4):
                    if (b, kq) not in a_ts:
                        load_a(b, kq)
                ps = [
                    ps1.tile([P, 2, 512], F32, tag="ps", name=f"ps_{b}_{ci}")
                    for ci in range(S)
                ]
                for k in range(KT1):
                    akt = a_ts[(b, k // 8)][:, k % 8, :]
                    for ci in range(S):
                        lhsT = akt[:, ci * 128 : (ci + 1) * 128]
                        for h in range(2):
                            nc.tensor.matmul(
                                ps[ci][:, h, :],
                                lhsT,
                                w_cs[k // 4][:, k % 4, h * 512 : (h + 1) * 512],
                                start=(k == 0),
                                stop=(k == KT1 - 1),
                            )
                for ci in range(S):
                    xe = xepool.tile([P, D_OUT], F16, tag="xe")
                    for h in range(2):
                        nc.vector.tensor_tensor(
                            xe[:, h * 512 : (h + 1) * 512],
                            ps[ci][:, h, :],
                            bias_sb[:, h * 512 : (h + 1) * 512],
                            mybir.AluOpType.add,
                        )
                    nc.gpsimd.dma_start(xh3[b][:, ci, :], xe[:])
                # AllGather this token block as soon as it's evicted.
                nc.gpsimd.collective_compute(
                    "AllGather",
                    mybir.AluOpType.bypass,
                    replica_groups=rg,
                    ins=[xh[b].ap().opt()],
                    outs=[xg[b].ap().opt()],
                )

        # ---------------- phase 3: combine GEMM ----------------
        # K-column-major: one k-tile column (all 8 experts) at a time, in
        # AllGather-block order, so each sub-section only depends on the
        # collectives that have already finished. SBUF-accumulated partial
        # outputs; fp16 operands, fp32 PSUM accumulate.
        with (
            tc.tile_pool(name="xkpool", bufs=16) as xkpool,
            tc.tile_pool(name="ckpool", bufs=16) as ckpool,
            tc.tile_pool(name="accpool", bufs=1) as accpool,
            tc.tile_pool(name="ps2", bufs=2, space="PSUM") as ps2,
        ):
            acc = accpool.tile([P, 4, 2, D_OUT], DT)
            koff = [sum(BLOCKS[:b]) for b in range(NB)]  # block k-tile offsets
            for kk in range(8):  # k-tile column within each expert
                b = max(bb for bb in range(NB) if koff[bb] <= kk)
                kt = kk - koff[b]
                S = BLOCKS[b]
                xk = []
                ck = []
                for j in range(E):
                    xt = xkpool.tile([P, D_OUT], F16, tag="xk", name=f"xk_{kk}_{j}")
                    nc.sync.dma_start(xt[:], xg3[b][:, j * S + kt, :])
                    xk.append(xt)
                    ct = ckpool.tile([P, C], F16, tag="ck", name=f"ck_{kk}_{j}")
                    nc.sync.dma_start(ct[:], cwt3[:, j * 8 + kk, :])
                    ck.append(ct)

                for tb in range(4):  # 256-token output blocks
                    pst = ps2.tile([P, 2, 2, 512], F32, tag="psc")
                    ps = [pst[:, 0], pst[:, 1]]
                    for j in range(E):
                        for ti in range(2):
                            lhsT = ck[j][
                                :, tb * 256 + ti * 128 : tb * 256 + (ti + 1) * 128
                            ]
                            for h in range(2):
                                nc.tensor.matmul(
                                    ps[ti][:, h, :],
                                    lhsT,
                                    xk[j][:, h * 512 : (h + 1) * 512],
                                    start=(j == 0),
                                    stop=(j == E - 1),
                                )
                    for ti in range(2):
                        for h in range(2):
                            dst = acc[:, tb, ti, h * 512 : (h + 1) * 512]
                            if kk == 0:
                                nc.vector.tensor_copy(dst, ps[ti][:, h, :])
                            else:
                                nc.vector.tensor_tensor(
                                    dst, ps[ti][:, h, :], dst, mybir.AluOpType.add
                                )
                    if kk == 7:
                        # stream this output block out while later blocks finish
                        nc.sync.dma_start(out4[:, tb, :, :], acc[:, tb, :, :])

    nc.compile()
    return nc


def _prep_inputs(expert_input, weight, bias, combine_weights):
    f32 = np.float32
    in_maps = []
    for e in range(E):
        in_maps.append(
            {
                "at": np.ascontiguousarray(expert_input[e].T, dtype=np.float16),
                "w": np.ascontiguousarray(weight[e], dtype=np.float16),
                "bias": np.ascontiguousarray(bias[e].reshape(1, D_OUT), dtype=f32),
                "cwt": np.ascontiguousarray(
                    combine_weights[e * C : (e + 1) * C, :].T, dtype=np.float16
                ),
            }
        )
    return in_maps


def _run(expert_input, weight, bias, combine_weights, trace=False):
    from concourse import bass_utils

    global _cached
    if _cached is None:
        _cached = _build()
    nc = _cached
    in_maps = _prep_inputs(expert_input, weight, bias, combine_weights)
    r = bass_utils.run_bass_kernel_spmd(
        nc, in_maps, core_ids=list(range(E)), trace=trace
    )
    output = np.concatenate([r.results[e]["out"] for e in range(E)], axis=0)
    return output.astype(np.float32, copy=False), r


def kernel(expert_input, weight, bias, combine_weights):
    output, _ = _run(expert_input, weight, bias, combine_weights)
    return output

